# revision 23
# baseline (speedup 1.0000x reference)
"""Cross-attention kernel for Trainium2 (8 NeuronCores, SPMD data-parallel).

Problem: B=4, C=128, 64x64 spatial (N=4096 tokens), 4 heads of dim 32.
  q = Wq @ query; k = Wk @ key; v = Wv @ key   (1x1 convs == channel matmuls)
  out = softmax(q^T k / sqrt(32)) @ v          (per batch*head)

Sharding: 16 (batch, head) jobs -> 2 per core. Core i handles batch i//2,
heads {2*(i%2), 2*(i%2)+1} i.e. output channels [64*(i%2), 64*(i%2)+64).

Structure (per core):
  - All projections are folded into the host-side input prep (they are tiny
    vs. the N^2 attention): tz_h = (log2e/sqrt(32) * Wk_h^T Wq_h) @ qin and
    vt_h = [kin^T Wv_h^T | 1] stream in as bf16 inputs, so the device runs
    only the N^2 pipeline: QK matmul -> exp -> PV matmul.
  - QK: scoresT[nk_chunk=128, nq_block=512] = kin_chunk(lhsT) @ tz_block,
    K=128 contraction, bf16 (raw kin is the stationary operand; the k
    projection is folded into tz via M = Wk^T Wq). Scores arrive in PSUM in
    the log2 domain.
  - exp: PSUM->SBUF drain split between ACT (exact table exp, scale=ln2)
    and DVE (one-op Schraudolph exp2: int16 <- y*128 + bias, bits
    reinterpreted as bf16). Tiles alternate A,D strictly inside a block (PV
    consumes exp results in order, so same-engine neighbors serialize); the
    per-block surplus ACT pair sits at the block boundary where PV chains
    are independent.
  - PV flipped: ctx[nq=128, 33] += probsT_chunk(lhsT, stationary) @ v_aug
    (moving: 32 v columns + the ones column -> softmax denominator), so each
    chunk's matmul streams only 33 columns instead of 512.
  - host: normalize by the denominator column and transpose to [C, N].
"""

import functools
import math

import numpy as np

NCORES = 8
B, C, HS, WS = 4, 128, 64, 64
N = HS * WS  # 4096 tokens
NUM_HEADS = 4
DH = 32  # head dim
HPC = 2  # heads per core

NQB = 512  # nq per QK matmul (one PSUM bank of f32)
NKC = 128  # nk chunk (PV contraction tile)
N_BLOCKS = N // NQB  # 8
N_CHUNKS = N // NKC  # 32
VTW = 33  # v^T tile width: 32 v cols + 1 ones col (denominator)

# Schraudolph exp2 in bf16: i16 = cvt(y*128 + (16256 - C)); bits = bf16 ~ 2^y
EXP2_A = 128.0
EXP2_B = 16256.0 - 5.25

# Strict D,A alternation: PV consumes exp results in order, so same-engine
# neighbors serialize; the steady-state period is max(ACT, DVE)/2 per pair.
# DVE (the wall engine) leads so it starts earliest; ACT takes the final
# tile and the per-block ob copy in its slack.
EXP_PAT = ["A", "D"] * 8



def _f32(x):
    return np.ascontiguousarray(np.asarray(x, dtype=np.float32))


def _bf16(x):
    import ml_dtypes

    return np.ascontiguousarray(
        np.asarray(x, dtype=np.float32).astype(ml_dtypes.bfloat16)
    )


@functools.lru_cache(maxsize=1)
def _build_program():
    from contextlib import ExitStack

    import concourse.tile as tile
    from concourse import bacc, mybir
    from concourse.bass import ts

    f32 = mybir.dt.float32
    bf16 = mybir.dt.bfloat16
    i16 = mybir.dt.int16
    AF = mybir.ActivationFunctionType
    ALU = mybir.AluOpType

    nc = bacc.Bacc(
        "TRN2",
        target_bir_lowering=False,
        debug=False,
        enable_asserts=False,
        num_devices=NCORES,
    )

    kin = nc.dram_tensor("kin", [128, N], bf16, kind="ExternalInput").ap()
    tzi = [
        nc.dram_tensor(f"tz{h}", [128, N], bf16, kind="ExternalInput").ap()
        for h in range(HPC)
    ]
    vti = [
        nc.dram_tensor(f"vt{h}", [128, VTW * N_CHUNKS], bf16, kind="ExternalInput").ap()
        for h in range(HPC)
    ]

    # per (h, nq-block): ctx rows [nq=128 x 4 j-tiles], cols 32 ctx + 1 den
    out_ctx = nc.dram_tensor(
        "out_ctx", [HPC * N_BLOCKS, 128, 4 * VTW], f32, kind="ExternalOutput"
    ).ap()

    ln2 = math.log(2.0)

    with tile.TileContext(nc) as tc, ExitStack() as ctx:
        persist = ctx.enter_context(tc.tile_pool(name="persist", bufs=1))

        kin_sb = persist.tile([128, N], bf16)
        tz = [persist.tile([128, N], bf16, name=f"tzs{h}") for h in range(HPC)]
        vt = [
            persist.tile([128, VTW * N_CHUNKS], bf16, name=f"vts{h}")
            for h in range(HPC)
        ]
        # load order: small first slices of kin/tz0 + vt gate the pipeline
        nc.sync.dma_start(out=kin_sb[:, 0:512], in_=kin[:, 0:512])
        nc.sync.dma_start(out=tz[0][:, 0:512], in_=tzi[0][:, 0:512])
        nc.sync.dma_start(out=vt[0], in_=vti[0])
        nc.sync.dma_start(out=kin_sb[:, 512:1024], in_=kin[:, 512:1024])
        nc.sync.dma_start(out=vt[1], in_=vti[1])
        nc.sync.dma_start(out=tz[0][:, 512:1024], in_=tzi[0][:, 512:1024])
        for t in range(1, 4):
            nc.sync.dma_start(
                out=kin_sb[:, ts(t, N // 4)], in_=kin[:, ts(t, N // 4)]
            )
        for t in range(1, 4):
            nc.sync.dma_start(
                out=tz[0][:, ts(t, N // 4)], in_=tzi[0][:, ts(t, N // 4)]
            )
        for t in range(4):
            nc.sync.dma_start(
                out=tz[1][:, ts(t, N // 4)], in_=tzi[1][:, ts(t, N // 4)]
            )

        scratch = persist.tile([128, NQB], bf16, name="scratch")
        nc.gpsimd.memset(scratch, 0.5)

        sc_pool = ctx.enter_context(tc.tile_pool(name="sc", bufs=3, space="PSUM"))
        ctx_pool = ctx.enter_context(tc.tile_pool(name="ctxp", bufs=2, space="PSUM"))
        ex_pool = ctx.enter_context(tc.tile_pool(name="ex", bufs=10))
        ob_pool = ctx.enter_context(tc.tile_pool(name="obp", bufs=4))

        # ---- PE clock warmup: keep the tensor engine busy from t~0.7us so
        # the 3us p-state ramp completes before the real QK stream needs full
        # speed (results are never read; the first real matmul's start=True
        # clears the bank). ----
        dsc = sc_pool.tile([128, 2 * NQB], f32, name="dsc", tag="sc")
        with tc.high_priority():
            for w in range(6):
                nc.tensor.matmul(
                    out=dsc[:, ts(w % 2, NQB)],
                    lhsT=scratch[:, 0:128],
                    rhs=scratch,
                    start=True,
                    stop=True,
                )

        # ---- attention ----
        for h in range(HPC):
            for b in range(N_BLOCKS):
                ctx_ps = ctx_pool.tile([128, 4 * VTW], f32, name="ctx_ps")
                for cc in range(N_CHUNKS // 2):
                    sc = sc_pool.tile([128, 2 * NQB], f32, name="sc", tag="sc")
                    with tc.high_priority(offset=320):
                        for u in range(2):
                            c = 2 * cc + u
                            nc.tensor.matmul(
                                out=sc[:, ts(u, NQB)],
                                lhsT=kin_sb[:, ts(c, NKC)],
                                rhs=tz[h][:, ts(b, NQB)],
                                start=True,
                                stop=True,
                            )
                    ex = ex_pool.tile([128, 2 * NQB], bf16, name="ex")
                    if EXP_PAT[cc] == "A":
                        nc.scalar.activation(ex, sc, AF.Exp, scale=ln2)
                    else:
                        nc.vector.tensor_scalar(
                            ex.bitcast(i16), sc, EXP2_A, EXP2_B,
                            op0=ALU.mult, op1=ALU.add,
                        )
                    for u in range(2):
                        c = 2 * cc + u
                        for j in range(4):
                            # NOTE: start=True clears has_written BANK-wide,
                            # so only the tile's very first matmul may set it
                            # (the bit-clear makes every region's first write
                            # an overwrite, later writes accumulate).
                            nc.tensor.matmul(
                                out=ctx_ps[:, ts(j, VTW)],
                                lhsT=ex[:, NQB * u + NKC * j : NQB * u + NKC * (j + 1)],
                                rhs=vt[h][:, ts(c, VTW)],
                                start=(c == 0 and j == 0),
                                stop=(c == N_CHUNKS - 1 and j == 3),
                                skip_group_check=True,
                            )
                ob = ob_pool.tile([128, 4 * VTW], f32, name="ob")
                nc.scalar.copy(ob, ctx_ps)
                nc.sync.dma_start(out=out_ctx[h * N_BLOCKS + b], in_=ob)

    nc.compile()
    return nc


def _shard_inputs(query, key, Wq, Wk, Wv):
    query = _f32(query).reshape(B, C, N)
    key = _f32(key).reshape(B, C, N)
    Wq, Wk, Wv = _f32(Wq), _f32(Wk), _f32(Wv)

    scale = math.log2(math.e) / math.sqrt(DH)
    in_maps = []
    for core in range(NCORES):
        b, half = core // 2, core % 2
        im = {"kin": _bf16(key[b])}
        for hl in range(HPC):
            ch0 = 64 * half + 32 * hl
            wq_h = Wq[ch0 : ch0 + 32, :]  # [32, 128]
            wk_h = Wk[ch0 : ch0 + 32, :]
            wv_h = Wv[ch0 : ch0 + 32, :]
            m = scale * (wk_h.T @ wq_h)  # [128, 128]
            im[f"tz{hl}"] = _bf16(m @ query[b])
            vt = np.ones((N, VTW), np.float32)
            vt[:, :32] = key[b].T @ wv_h.T
            im[f"vt{hl}"] = _bf16(
                vt.reshape(N_CHUNKS, NKC, VTW)
                .transpose(1, 0, 2)
                .reshape(NKC, N_CHUNKS * VTW)
            )
        in_maps.append(im)
    return in_maps


def _run(in_maps, trace=False):
    from concourse import bass_utils

    nc = _build_program()
    return bass_utils.run_bass_kernel_spmd(
        nc, in_maps, core_ids=list(range(NCORES)), trace=trace
    )


def _assemble(results):
    out = np.empty((B, C, N), np.float32)
    for core in range(NCORES):
        b, half = core // 2, core % 2
        r = results[core]
        t = np.asarray(r["out_ctx"], np.float32)  # [16, 128, 132]
        t = t.reshape(HPC, N_BLOCKS, 128, 4, VTW)
        ctx = t[..., :32]  # [h, b8, p, j, d]
        den = t[..., 32]  # [h, b8, p, j]
        # nq index = b8*512 + j*128 + p -> order (b8, j, p)
        ctx = np.transpose(ctx, (0, 1, 3, 2, 4)).reshape(HPC, N, 32)
        den = np.transpose(den, (0, 1, 3, 2)).reshape(HPC, N)
        for hl in range(HPC):
            ch0 = 64 * half + 32 * hl
            out[b, ch0 : ch0 + 32, :] = (ctx[hl] / den[hl][:, None]).T
    return out.reshape(B, C, HS, WS)


def kernel(query, key, Wq, Wk, Wv):
    in_maps = _shard_inputs(query, key, Wq, Wk, Wv)
    res = _run(in_maps)
    return _assemble(res.results)
